# revision 1
# baseline (speedup 1.0000x reference)
"""Causal self-attention (B=4, T=2048, C=2048, H=16) on 8 trn2 NeuronCores.

Sharding: tensor-parallel over heads — 2 heads per core. Every core gets the
full (pre-transposed) activation xT, its 2 heads' slice of Wqkv columns and
Wproj rows, computes a full [B*T, C] partial output (fp16), and the host sums
the 8 partials (the "all-reduce after output projection" done host-side).

Per-core dataflow (all matmuls fp16 on PE):
  xT tiles --DMA--> QKV proj -> Q^T,K^T [d,t] + V [t,d]
  S = Q^T.T @ K^T chunks (PSUM f32) -> +causal mask -> exp (ACT) -> P (fp16)
  y^T = sum_k V_k^T-block @ P^T-block (PSUM f32, accumulated over k-blocks)
  softmax denominator: DVE reduce of P over k-blocks -> one ones-matmul ->
  reciprocal_approx_fast -> PE row-broadcast -> DVE normalize.
  The den/rec/normalize epilogue is software-pipelined two attention units
  deep so the in-order PE queue never waits on the DVE/ACT chain (a PE stall
  also drops the PE to its half-speed p-state for ~3us).
  out_partial = y^T.T @ Wproj-rows (accumulate 2 head-chunks) -> fp16 -> DMA
"""
import numpy as np

B, T, C = 4, 2048, 2048
H, HD = 16, 128
N_CORES = 8
HPC = H // N_CORES          # heads per core = 2
SCALE = float(1.0 / np.sqrt(HD))
NEG = -1e9

_CACHE = {}


def _build_nc():
    import concourse.bass as bass
    from concourse import bacc
    import concourse.tile as tile
    import concourse.mybir as mybir
    from concourse.masks import make_identity
    from contextlib import ExitStack

    f32 = mybir.dt.float32
    f16 = mybir.dt.float16
    Exp = mybir.ActivationFunctionType.Exp
    AXX = mybir.AxisListType.X
    Add = mybir.AluOpType.add

    nc = bacc.Bacc("TRN2", target_bir_lowering=False, debug=False,
                   enable_asserts=True, num_devices=N_CORES)

    # Inputs (per-core shards prepared on host)
    xT = nc.dram_tensor("xt", [C, B * T], f16, kind="ExternalInput").ap()
    wqkv = nc.dram_tensor("wqkv", [C, 6 * HD], f16, kind="ExternalInput").ap()
    wproj = nc.dram_tensor("wproj", [HPC * HD, C], f16, kind="ExternalInput").ap()
    out = nc.dram_tensor("out", [B * T, C], f16, kind="ExternalOutput").ap()

    # DRAM views: c-chunked weights
    wqkv_v = wqkv.rearrange("(cc p) (jj d) -> p cc jj d", p=128, d=HD)  # [128,16,6,128]
    wproj_v = wproj.rearrange("(jh p) c -> p jh c", p=128)              # [128,2,2048]

    NCC = C // 128        # 16 contraction chunks
    NTCH = T // 512       # 4 t-chunks per batch

    with tile.TileContext(nc) as tc, ExitStack() as ctx:
        const = ctx.enter_context(tc.tile_pool(name="const", bufs=1))
        wpool = ctx.enter_context(tc.tile_pool(name="w", bufs=1))
        xtp = ctx.enter_context(tc.tile_pool(name="xt", bufs=2))
        qkvp = ctx.enter_context(tc.tile_pool(name="qkv", bufs=2))

        dnp = ctx.enter_context(tc.tile_pool(name="dn", bufs=2))
        rp = ctx.enter_context(tc.tile_pool(name="r", bufs=2))
        ptp = ctx.enter_context(tc.tile_pool(name="pt", bufs=2))
        ytp = ctx.enter_context(tc.tile_pool(name="yt", bufs=2))
        op = ctx.enter_context(tc.tile_pool(name="o", bufs=10))
        psA = ctx.enter_context(tc.tile_pool(name="psA", bufs=4, space="PSUM"))
        psV = ctx.enter_context(tc.tile_pool(name="psV", bufs=3, space="PSUM"))
        psT = ctx.enter_context(tc.tile_pool(name="psT", bufs=1, space="PSUM"))

        ident_f = const.tile([128, 128], f32)
        make_identity(nc, ident_f)
        ident_h = const.tile([128, 128], f16)
        nc.scalar.copy(ident_h, ident_f)
        # causal mask as a PE accumulation: st[k,q] += -60000 * (k > q).
        # maskL = -60000*I (stationary), maskU[c,q] = 1 where c > q (moving);
        # keeps the mask on the Tensor engine - no DVE hop in the S->exp chain
        maskL = const.tile([128, 128], f16)
        nc.scalar.mul(maskL, ident_f, -60000.0)
        mU32 = const.tile([128, 128], f32)
        nc.gpsimd.memset(mU32, 1.0)
        nc.gpsimd.affine_select(
            out=mU32, in_=mU32, compare_op=mybir.AluOpType.is_ge, fill=0.0,
            base=-1, pattern=[[-1, 128]], channel_multiplier=1)
        maskU = const.tile([128, 128], f16)
        nc.scalar.copy(maskU, mU32)
        ones_col = const.tile([128, 1], f16)
        nc.vector.memset(ones_col, 1.0)
        ones_row = const.tile([1, 128], f16)
        nc.vector.memset(ones_row, 1.0)

        w_sb = wpool.tile([128, NCC, 6, HD], f16)
        wp_sb = wpool.tile([128, 2, C], f16)

        def qkv_dma(b, tch):
            t0 = b * T + tch * 512
            xt_t = xtp.tile([128, NCC, 512], f16, tag="xt",
                            name=f"xt{b}{tch}")
            for cc in range(NCC):
                nc.sync.dma_start(
                    xt_t[:, cc, :], xT[cc * 128:(cc + 1) * 128, t0:t0 + 512])
            return xt_t

        def qkv_mm(b, tch, xt_t, qkv_tiles):
            qt, kt, vt, v = qkv_tiles
            for jj in range(6):  # q_h0, q_h1, k_h0, k_h1, v_h0, v_h1
                qk_ps = psA.tile([128, 512], f32, tag="psA")
                for cc in range(NCC):
                    nc.tensor.matmul(qk_ps, w_sb[:, cc, jj, :], xt_t[:, cc, :],
                                     start=(cc == 0), stop=(cc == NCC - 1))
                dst = (qt, qt, kt, kt, vt, vt)[jj]
                # DVE, not ACT: keeps the scalar engine free for the exp
                # stream that gates the in-flight PV matmuls
                nc.vector.tensor_copy(
                    dst[:, jj % 2, tch * 512:(tch + 1) * 512], qk_ps)
            # transpose this chunk's V^T slice -> V [t, d]
            for hh in range(HPC):
                for tb in range(4):
                    tg = tch * 4 + tb
                    vp = psT.tile([128, 128], f16, tag="psT")
                    nc.tensor.transpose(
                        vp, vt[:, hh, tg * 128:(tg + 1) * 128], ident_h)
                    nc.vector.tensor_copy(v[:, tg, hh * HD:(hh + 1) * HD], vp)

        # Global S->exp->PV pipeline, 2 S-blocks deep ACROSS unit boundaries:
        # a unit's tail PV matmuls are covered by the next unit's (or the
        # next QKV chunk's) S matmuls, so the exp latency never exposes the
        # in-order PE queue.
        PIPE = []

        def pipe_flush():
            kb, qs, st, pt_sb, den_f, yt_ps, v_ap, nkb = PIPE.pop(0)
            nc.scalar.activation(
                pt_sb[:, kb, qs:512], st[:, qs:512], Exp, scale=SCALE)
            # running denominator partial sum (per k-partition) on DVE,
            # chasing the exp stream
            if kb == 0:
                nc.vector.tensor_copy(den_f, pt_sb[:, 0, :])
            else:
                nc.vector.tensor_add(
                    den_f[:, qs:512], den_f[:, qs:512], pt_sb[:, kb, qs:512])
            nc.tensor.matmul(
                yt_ps[:, qs:512], v_ap, pt_sb[:, kb, qs:512],
                start=(kb == 0), stop=(kb == nkb - 1))

        def pipe_push(entry):
            PIPE.append(entry)
            if len(PIPE) > 2:
                pipe_flush()

        def pipe_drain():
            while PIPE:
                pipe_flush()

        def emit_attn_mm(b, qg, h, qkv_tiles):
            """S matmuls + exp + PV accumulation for one (batch, q-group,
            head) unit. The softmax epilogue is deferred (see emit_epi*)."""
            qt, kt, vt, v = qkv_tiles
            pt_sb = ptp.tile([128, T // 128, 512], f16, tag="pt")
            den_f = dnp.tile([128, 512], f32, tag="den")
            yt_ps = psV.tile([128, 512], f32, tag="psV")
            nkb = 4 * qg + 4
            for kb in range(nkb):
                kk = kb - 4 * qg
                qs = max(0, kk) * 128
                st = psA.tile([128, 512], f32, tag="psA")
                nc.tensor.matmul(
                    st[:, qs:512], kt[:, h, kb * 128:(kb + 1) * 128],
                    qt[:, h, qg * 512 + qs:(qg + 1) * 512],
                    start=True, stop=(kk < 0))
                if kk >= 0:
                    nc.tensor.matmul(
                        st[:, qs:qs + 128], maskL, maskU,
                        start=False, stop=True)
                pipe_push((kb, qs, st, pt_sb, den_f, yt_ps,
                           v[:, kb, h * HD:(h + 1) * HD], nkb))
            return {"b": b, "qg": qg, "h": h, "den_f": den_f, "yt_ps": yt_ps,
                    "nkb": nkb}

        def emit_epiA(u):
            """Denominator: one ones-matmul for the partition reduction of
            the DVE-accumulated partial sums, then fast reciprocal. Emitted
            one unit after u's matmuls so it overlaps the next unit's PE
            work."""
            den16 = dnp.tile([128, 512], f16, tag="den16")
            nc.vector.tensor_copy(den16, u["den_f"])
            den_row = psA.tile([1, 512], f32, tag="psA")
            nc.tensor.matmul(den_row, ones_col, den16, start=True, stop=True)
            rec_sb = dnp.tile([1, 512], f32, tag="rec")
            nc.vector.reciprocal_approx_fast(rec_sb, den_row[0:1, :])
            rec16 = dnp.tile([1, 512], f16, tag="rec16")
            nc.scalar.copy(rec16, rec_sb)
            u["rec16"] = rec16

        def emit_epiB(u, yt):
            """Broadcast 1/den across partitions (PE) and normalize y^T.
            Emitted two units after u's matmuls: the reciprocal has had a
            full unit of slack, so the PE does not stall on the DVE chain."""
            r_ps = psA.tile([128, 512], f32, tag="psA")
            nc.tensor.matmul(r_ps, ones_row, u["rec16"], start=True, stop=True)
            r_sb = rp.tile([128, 512], f32, tag="rsb")
            nc.scalar.copy(r_sb, r_ps)
            nc.vector.tensor_mul(yt[:, u["h"], :], u["yt_ps"], r_sb)

        def emit_proj(b, qg, yt):
            for tt in range(4):
                for co in range(4):
                    o_ps = psA.tile([128, 512], f32, tag="psA")
                    for jh in range(HPC):
                        nc.tensor.matmul(
                            o_ps, yt[:, jh, tt * 128:(tt + 1) * 128],
                            wp_sb[:, jh, co * 512:(co + 1) * 512],
                            start=(jh == 0), stop=(jh == HPC - 1))
                    o_sb = op.tile([128, 512], f16, tag="osb")
                    # alternate PSUM evacuation between DVE and ACT so
                    # neither becomes the PSUM ring's bottleneck
                    if (tt * 4 + co) % 2 == 0:
                        nc.vector.tensor_copy(o_sb, o_ps)
                    else:
                        nc.scalar.copy(o_sb, o_ps)
                    r0 = b * T + qg * 512 + tt * 128
                    nc.sync.dma_start(
                        out[r0:r0 + 128, co * 512:(co + 1) * 512], o_sb)

        def alloc_qkv_tiles():
            qt = qkvp.tile([128, HPC, T], f16, tag="qt")
            kt = qkvp.tile([128, HPC, T], f16, tag="kt")
            vt = qkvp.tile([128, HPC, T], f16, tag="vt")
            v = qkvp.tile([128, T // 128, HPC * HD], f16, tag="v")
            return (qt, kt, vt, v)

        # Pipeline: QKV chunks of batch b+1 interleave into batch b's
        # attention stream, with each chunk's xt DMA issued one chunk ahead
        # so the PE never waits on an in-flight transfer; softmax epilogues
        # trail their unit by 1 (epiA) and 2 (epiB) units so the PE never
        # waits on DVE/ACT results.
        chunk_after = {}
        _seq = [(b, t) for b in range(B) for t in range(NTCH)]
        for _i, _c in enumerate(_seq[:-1]):
            chunk_after[_c] = _seq[_i + 1]

        tiles = alloc_qkv_tiles()
        xt_pend = {}
        # startup: interleave weight-chunk and first-xt-chunk DMA issue so
        # the first matmul waits on two small transfers, not all of them
        xt00 = xtp.tile([128, NCC, 512], f16, tag="xt", name="xt00")
        xt_pend[(0, 0)] = xt00
        for cc in range(NCC):
            nc.sync.dma_start(w_sb[:, cc, :, :], wqkv_v[:, cc, :, :])
            nc.sync.dma_start(
                xt00[:, cc, :], xT[cc * 128:(cc + 1) * 128, 0:512])

        def run_chunk(bt):
            if bt in chunk_after:
                nb = chunk_after[bt]
                xt_pend[nb] = qkv_dma(*nb)
            qkv_mm(bt[0], bt[1], xt_pend.pop(bt),
                   tiles if bt[0] == cur_b else nxt)

        cur_b = 0
        nxt = tiles
        run_chunk((0, 0))
        nc.sync.dma_start(wp_sb, wproj_v)
        for tch in range(1, NTCH):
            run_chunk((0, tch))
        prevA = None   # unit awaiting epiA
        prevB = None   # unit awaiting epiB
        yts = {}       # (b, qg) -> yt tile
        for b in range(B):
            cur_b = b
            nxt = alloc_qkv_tiles() if b + 1 < B else None
            for qg in range(4):
                for h in range(HPC):
                    if h == 0 and nxt is not None:
                        run_chunk((b + 1, qg))
                    u = emit_attn_mm(b, qg, h, tiles)
                    if h == 0:
                        yts[(b, qg)] = ytp.tile(
                            [128, HPC, 512], f16, tag="yt", name=f"yt{b}{qg}")
                    if prevA is not None:
                        emit_epiA(prevA)
                    if prevB is not None:
                        emit_epiB(prevB, yts[(prevB["b"], prevB["qg"])])
                        if prevB["h"] == 1:
                            emit_proj(prevB["b"], prevB["qg"],
                                      yts.pop((prevB["b"], prevB["qg"])))
                    prevB = prevA
                    prevA = u
            tiles = nxt
        # drain the epilogue pipeline
        pipe_drain()
        emit_epiA(prevA)
        emit_epiB(prevB, yts[(prevB["b"], prevB["qg"])])
        if prevB["h"] == 1:
            emit_proj(prevB["b"], prevB["qg"],
                      yts.pop((prevB["b"], prevB["qg"])))
        emit_epiB(prevA, yts[(prevA["b"], prevA["qg"])])
        if prevA["h"] == 1:
            emit_proj(prevA["b"], prevA["qg"],
                      yts.pop((prevA["b"], prevA["qg"])))

    nc.compile()
    return nc


def _get_nc():
    if "nc" not in _CACHE:
        _CACHE["nc"] = _build_nc()
    return _CACHE["nc"]


def _make_in_maps(x2d, Wqkv, Wproj):
    xT = np.ascontiguousarray(x2d.T).astype(np.float16)  # [C, B*T]
    in_maps = []
    for c in range(N_CORES):
        h0 = c * HPC
        cols = []
        for part in range(3):  # q, k, v blocks of Wqkv columns
            for h in range(HPC):
                j0 = part * C + (h0 + h) * HD
                cols.append(Wqkv[:, j0:j0 + HD])
        wq = np.ascontiguousarray(np.concatenate(cols, axis=1)).astype(np.float16)
        wp = np.ascontiguousarray(
            Wproj[h0 * HD:(h0 + HPC) * HD, :]).astype(np.float16)
        in_maps.append({"xt": xT, "wqkv": wq, "wproj": wp})
    return in_maps


def run_shards(in_maps, trace=False):
    from concourse.bass_utils import run_bass_kernel_spmd
    nc = _get_nc()
    last_err = None
    for _attempt in range(3):
        try:
            return run_bass_kernel_spmd(
                nc, in_maps, core_ids=list(range(N_CORES)), trace=trace)
        except Exception as e:  # transient NRT device errors — retry
            last_err = e
            if "UNAVAILABLE" not in str(e) and "UNRECOVERABLE" not in str(e):
                raise
    raise last_err


def kernel(x, Wqkv, Wproj):
    x = np.asarray(x, dtype=np.float32)
    Wqkv = np.asarray(Wqkv, dtype=np.float32)
    Wproj = np.asarray(Wproj, dtype=np.float32)
    x2d = np.ascontiguousarray(x.reshape(B * T, C))

    in_maps = _make_in_maps(x2d, Wqkv, Wproj)
    res = run_shards(in_maps)

    acc = res.results[0]["out"].astype(np.float32)
    for c in range(1, N_CORES):
        acc += res.results[c]["out"].astype(np.float32)
    return acc.reshape(B, T, C)



# revision 3
# speedup vs baseline: 1.1174x; 1.1174x over previous
"""Causal self-attention (B=4, T=2048, C=2048, H=16) on 8 trn2 NeuronCores.

Sharding: tensor-parallel over heads - 2 heads per core. Every core gets the
full (pre-transposed) activation xT, its 2 heads' slice of Wqkv columns and
Wproj rows, computes a full [B*T, C] partial output (fp16), and the host sums
the 8 partials (the "all-reduce after output projection" done host-side).

Per-core dataflow (all matmuls fp16 on PE), v2 schedule:
  * Q^T,K^T [d,t] via W-stationary matmuls; V [t,d] computed DIRECTLY via
    x^T-stationary matmuls (no PE transposes).
  * S = K^T-block.T @ Q^T chunks (PSUM f32) -> exp (ACT) -> causal zeroing of
    diagonal blocks on GPSIMD (affine_select, off the PE) -> P (fp16).
  * softmax denominator chased on DVE in fp16; partition-reduced by one
    ones-column matmul; reciprocal broadcast by one ones-row matmul.
  * The whole emission is a single interleaved stream: QKV matmuls of the
    NEXT chunk and output-projection matmuls of the PREVIOUS (b,qg) group are
    woven between attention S/PV blocks, so the in-order PE queue always has
    exp-independent work while ACT streams the exponentials.
  * PSUM budget (8 banks): S-blocks ring 3, QKV accum 1, PV accum ring 2,
    shared transient ring 2 (proj out / V accum / den row / recip bcast).
"""
import numpy as np

B, T, C = 4, 2048, 2048
H, HD = 16, 128
N_CORES = 8
HPC = H // N_CORES          # heads per core = 2
SCALE = float(1.0 / np.sqrt(HD))

_CACHE = {}


def _build_nc():
    import concourse.bass as bass
    from concourse import bacc
    import concourse.tile as tile
    import concourse.mybir as mybir
    from contextlib import ExitStack

    f32 = mybir.dt.float32
    f16 = mybir.dt.float16
    Exp = mybir.ActivationFunctionType.Exp
    IsGe = mybir.AluOpType.is_ge

    nc = bacc.Bacc("TRN2", target_bir_lowering=False, debug=False,
                   enable_asserts=True, num_devices=N_CORES)

    # Inputs (per-core shards prepared on host)
    xT = nc.dram_tensor("xt", [C, B * T], f16, kind="ExternalInput").ap()
    wqkv = nc.dram_tensor("wqkv", [C, 6 * HD], f16, kind="ExternalInput").ap()
    wproj = nc.dram_tensor("wproj", [HPC * HD, C], f16, kind="ExternalInput").ap()
    out = nc.dram_tensor("out", [B * T, C], f16, kind="ExternalOutput").ap()

    # DRAM views
    # j-major weight view: one DMA per qkv column-block j (j: q0,q1,k0,k1,v0,v1)
    wqkv_v = wqkv.rearrange("(cc p) (j d) -> p j cc d", p=128, d=HD)  # [128,6,16,128]
    wproj_v = wproj.rearrange("(jh p) c -> p jh c", p=128)            # [128,2,2048]
    xv = xT.rearrange("(cc p) t -> p cc t", p=128)                    # [128,16,8192]

    NCC = C // 128        # 16 contraction chunks
    SEQ = [(b, qg) for b in range(B) for qg in range(4)]

    with tile.TileContext(nc) as tc, ExitStack() as ctx:
        const = ctx.enter_context(tc.tile_pool(name="const", bufs=1))
        wpool = ctx.enter_context(tc.tile_pool(name="w", bufs=1))
        xtp = ctx.enter_context(tc.tile_pool(name="xt", bufs=2))
        qkvp = ctx.enter_context(tc.tile_pool(name="qkv", bufs=2))
        ptp = ctx.enter_context(tc.tile_pool(name="pt", bufs=2))
        dnp = ctx.enter_context(tc.tile_pool(name="dn", bufs=3))
        rp = ctx.enter_context(tc.tile_pool(name="r", bufs=2))
        ytp = ctx.enter_context(tc.tile_pool(name="yt", bufs=2))
        op = ctx.enter_context(tc.tile_pool(name="o", bufs=4))
        ps = ctx.enter_context(tc.tile_pool(name="ps", bufs=1, space="PSUM"))

        ones_col = const.tile([128, 1], f16)
        nc.vector.memset(ones_col, 1.0)
        ones_row = const.tile([1, 128], f16)
        nc.vector.memset(ones_row, 1.0)

        w_sb = wpool.tile([128, 6, NCC, HD], f16)
        wp_sb = wpool.tile([128, 2, C], f16)

        # ---------- chunk (QKV) machinery ----------
        xt_pend = {}
        sets = {}

        def dma_xt(c):
            b, qg = c
            t0 = b * T + qg * 512
            xt_t = xtp.tile([128, NCC, 512], f16, tag="xt", name=f"xt{b}{qg}")
            for g in range(4):
                nc.sync.dma_start(
                    xt_t[:, 4 * g:4 * g + 4, :], xv[:, 4 * g:4 * g + 4, t0:t0 + 512])
            xt_pend[c] = xt_t

        def alloc_set(b):
            qt = qkvp.tile([128, HPC, T], f16, tag="qt", name=f"qt{b}")
            kt = qkvp.tile([128, HPC, T], f16, tag="kt", name=f"kt{b}")
            v = qkvp.tile([128, T // 128, HPC * HD], f16, tag="v", name=f"v{b}")
            sets[b] = (qt, kt, v)

        def emit_qk_lump(c, j, kick):
            """16 W-stationary matmuls: one of q_h0/q_h1/k_h0/k_h1 for chunk c."""
            b, qg = c
            if b not in sets:
                alloc_set(b)
            qt, kt, v = sets[b]
            xt_t = xt_pend[c]
            qk_ps = ps.tile([128, 512], f32, tag="qk", bufs=1)
            for cc in range(NCC):
                nc.tensor.matmul(qk_ps, w_sb[:, j, cc, :], xt_t[:, cc, :],
                                 start=(cc == 0), stop=(cc == NCC - 1))
                if cc % 4 == 3:
                    kick()
            dst = (qt, qt, kt, kt)[j]
            nc.vector.tensor_copy(dst[:, j % 2, qg * 512:(qg + 1) * 512], qk_ps)

        def emit_v_lump(c, tb, kick, last=False):
            """16 x^T-stationary matmuls: V[t-block, 2*HD] for chunk c, direct
            [t, d] layout - no transposes."""
            b, qg = c
            if b not in sets:
                alloc_set(b)
            qt, kt, v = sets[b]
            xt_t = xt_pend[c]
            v_ps = ps.tile([128, 2 * HD], f32, tag="ov", bufs=2)
            for cc in range(NCC):
                nc.tensor.matmul(
                    v_ps, xt_t[:, cc, tb * 128:(tb + 1) * 128],
                    w_sb[:, 4:6, cc, :],
                    start=(cc == 0), stop=(cc == NCC - 1))
                if cc % 4 == 3:
                    kick()
            nc.vector.tensor_copy(v[:, qg * 4 + tb, :], v_ps)

        # ---------- attention pipeline (software-pipelined, depth 2) ----------
        PIPE = []

        def pipe_flush():
            kb, qs, st, u = PIPE.pop(0)
            pt, den = u["pt"], u["den"]
            nc.scalar.activation(
                pt[:, kb, qs:512], st[:, qs:512], Exp, scale=SCALE)
            if kb - 4 * u["qg"] >= 0:
                # causal zeroing of the upper triangle of the diagonal
                # 128x128 sub-block - on GPSIMD, off the PE/ACT/DVE hot paths
                nc.gpsimd.affine_select(
                    out=pt[:, kb, qs:qs + 128], in_=pt[:, kb, qs:qs + 128],
                    compare_op=IsGe, fill=0.0,
                    base=0, pattern=[[1, 128]], channel_multiplier=-1)
            if kb == 0:
                nc.vector.tensor_copy(den, pt[:, 0, :])
            else:
                nc.vector.tensor_add(
                    den[:, qs:512], den[:, qs:512], pt[:, kb, qs:512])
            nc.tensor.matmul(
                u["yt_ps"][:, qs:512], u["v_ap"][:, kb, :],
                pt[:, kb, qs:512],
                start=(kb == 0), stop=(kb == u["nkb"] - 1))

        def kick():
            if PIPE:
                pipe_flush()

        def pipe_push(e):
            PIPE.append(e)
            while len(PIPE) > 2:
                pipe_flush()

        def make_unit(b, qg, h):
            qt, kt, v = sets[b]
            return {
                "b": b, "qg": qg, "h": h, "nkb": 4 * qg + 4,
                "pt": ptp.tile([128, T // 128, 512], f16, tag="pt",
                               name=f"pt{b}{qg}{h}"),
                "den": dnp.tile([128, 512], f16, tag="den", name=f"dn{b}{qg}{h}"),
                "yt_ps": ps.tile([128, 512], f32, tag="yt", bufs=2,
                                 name=f"ytps{b}{qg}{h}"),
                "v_ap": v[:, :, h * HD:(h + 1) * HD],
            }

        def emit_block(u, kb):
            b, qg, h = u["b"], u["qg"], u["h"]
            qt, kt, v = sets[b]
            kk = kb - 4 * qg
            qs = max(0, kk) * 128
            st = ps.tile([128, 512], f32, tag="st", bufs=3)
            nc.tensor.matmul(
                st[:, qs:512], kt[:, h, kb * 128:(kb + 1) * 128],
                qt[:, h, qg * 512 + qs:(qg + 1) * 512],
                start=True, stop=True)
            pipe_push((kb, qs, st, u))

        # ---------- softmax epilogue (trail-1) ----------
        def emit_epiA(u):
            den_row = ps.tile([1, 512], f32, tag="ov", bufs=2,
                              name=f"dr{u['b']}{u['qg']}{u['h']}")
            nc.tensor.matmul(den_row, ones_col, u["den"], start=True, stop=True)
            rec = rp.tile([1, 512], f32, tag="rec")
            nc.vector.reciprocal_approx_fast(rec, den_row[0:1, :])
            rec16 = rp.tile([1, 512], f16, tag="rec16")
            nc.scalar.copy(rec16, rec)
            u["rec16"] = rec16

        def emit_epiB(u, yt):
            r_ps = ps.tile([128, 512], f32, tag="ov", bufs=2,
                           name=f"rps{u['b']}{u['qg']}{u['h']}")
            nc.tensor.matmul(r_ps, ones_row, u["rec16"], start=True, stop=True)
            r_sb = rp.tile([128, 512], f32, tag="rsb")
            nc.scalar.copy(r_sb, r_ps)
            nc.vector.tensor_mul(yt[:, u["h"], :], u["yt_ps"], r_sb)

        # ---------- output projection ----------
        osb_pend = {}

        def emit_proj_pair(b, qg, yt, tt, co, single_dma):
            o_ps = ps.tile([128, 512], f32, tag="ov", bufs=2,
                           name=f"ops{b}{qg}{tt}{co}")
            for jh in range(HPC):
                nc.tensor.matmul(
                    o_ps, yt[:, jh, tt * 128:(tt + 1) * 128],
                    wp_sb[:, jh, co * 512:(co + 1) * 512],
                    start=(jh == 0), stop=(jh == HPC - 1))
            r0 = b * T + qg * 512 + tt * 128
            if single_dma:
                o_sb = op.tile([128, 512], f16, tag="osb1", bufs=2,
                               name=f"os{b}{qg}{tt}{co}")
                if (tt * 4 + co) % 2 == 0:
                    nc.vector.tensor_copy(o_sb, o_ps)
                else:
                    nc.scalar.copy(o_sb, o_ps)
                nc.sync.dma_start(
                    out[r0:r0 + 128, co * 512:(co + 1) * 512], o_sb)
                return
            if co % 2 == 0:
                osb_pend[(b, qg)] = op.tile(
                    [128, 1024], f16, tag="osb", bufs=4, name=f"os{b}{qg}{tt}{co}")
            o_sb = osb_pend[(b, qg)]
            dst = o_sb[:, (co % 2) * 512:(co % 2 + 1) * 512]
            # alternate PSUM evacuation between DVE and ACT
            if (tt * 4 + co) % 2 == 0:
                nc.vector.tensor_copy(dst, o_ps)
            else:
                nc.scalar.copy(dst, o_ps)
            if co % 2 == 1:
                c2 = co // 2
                nc.sync.dma_start(
                    out[r0:r0 + 128, c2 * 1024:(c2 + 1) * 1024], o_sb)

        # ---------- per-step interleaved emission ----------
        def emit_step(prev, idx, b, qg):
            nkb = 4 * qg + 4
            n2 = 2 * nkb
            ev = []
            seq_n = [0]

            def at(pos, fn):
                seq_n[0] += 1
                ev.append((pos, seq_n[0], fn))

            step_state = {"units": {}, "yt": None}

            def block_fn(h, kb):
                def f():
                    u = step_state["units"].get(h)
                    if u is None:
                        u = step_state["units"][h] = make_unit(b, qg, h)
                        if h == 0:
                            step_state["yt"] = ytp.tile(
                                [128, HPC, 512], f16, tag="yt",
                                name=f"yt{b}{qg}")
                    emit_block(u, kb)
                return f

            for h in range(2):
                for kb in range(nkb):
                    at(h * nkb + kb, block_fn(h, kb))

            if idx + 2 < len(SEQ):
                at(-1.0, (lambda c: lambda: dma_xt(c))(SEQ[idx + 2]))

            if idx + 1 < len(SEQ):
                nxt = SEQ[idx + 1]
                order = [("qk", 0), ("v", 0), ("qk", 2), ("v", 1),
                         ("qk", 1), ("v", 2), ("qk", 3), ("v", 3)]
                for i, (kind, j) in enumerate(order):
                    if kind == "qk":
                        fn = (lambda jj: lambda: emit_qk_lump(nxt, jj, kick))(j)
                    else:
                        fn = (lambda tb: lambda: emit_v_lump(
                            nxt, tb, kick, last=(tb == 3)))(j)
                    at((i + 0.45) * n2 / 8, fn)

            if prev is not None:
                pu, pyt = prev["h1"], prev["yt"]
                pb, pqg = prev["bqg"]
                at(2.4, (lambda u: lambda: emit_epiA(u))(pu))
                at(4.4, (lambda u, y: lambda: emit_epiB(u, y))(pu, pyt))
                span = max(n2 - 6, 2)
                for i in range(16):
                    tt, co = divmod(i, 4)
                    at(5.5 + i * span / 16.0,
                       (lambda t_, c_: lambda: emit_proj_pair(
                           pb, pqg, pyt, t_, c_, False))(tt, co))

            at(nkb + 2.4, lambda: emit_epiA(step_state["units"][0]))
            at(nkb + 4.4, lambda: emit_epiB(step_state["units"][0],
                                            step_state["yt"]))

            ev.sort(key=lambda e: (e[0], e[1]))
            for _, _, fn in ev:
                fn()
            return {"h1": step_state["units"][1], "yt": step_state["yt"],
                    "bqg": (b, qg)}

        # ---------- prologue: weights + first chunk ----------
        alloc_set(0)
        nc.sync.dma_start(w_sb[:, 0], wqkv_v[:, 0])
        dma_xt((0, 0))
        nc.sync.dma_start(w_sb[:, 2], wqkv_v[:, 2])
        nc.sync.dma_start(w_sb[:, 4], wqkv_v[:, 4])
        nc.sync.dma_start(w_sb[:, 5], wqkv_v[:, 5])
        nc.sync.dma_start(w_sb[:, 1], wqkv_v[:, 1])
        nc.sync.dma_start(w_sb[:, 3], wqkv_v[:, 3])
        dma_xt((0, 1))
        nop = lambda: None
        emit_qk_lump((0, 0), 0, nop)
        nc.sync.dma_start(wp_sb, wproj_v)
        emit_qk_lump((0, 0), 2, nop)
        for tb in range(4):
            emit_v_lump((0, 0), tb, nop, last=(tb == 3))
        emit_qk_lump((0, 0), 1, nop)
        emit_qk_lump((0, 0), 3, nop)

        # ---------- main loop ----------
        prev = None
        for idx, (b, qg) in enumerate(SEQ):
            prev = emit_step(prev, idx, b, qg)

        # ---------- drain ----------
        while PIPE:
            pipe_flush()
        emit_epiA(prev["h1"])
        emit_epiB(prev["h1"], prev["yt"])
        for i in range(16):
            tt, co = divmod(i, 4)
            emit_proj_pair(3, 3, prev["yt"], tt, co, True)

    nc.compile()
    return nc


def _get_nc():
    if "nc" not in _CACHE:
        _CACHE["nc"] = _build_nc()
    return _CACHE["nc"]


def _make_in_maps(x2d, Wqkv, Wproj):
    xT = np.ascontiguousarray(x2d.T).astype(np.float16)  # [C, B*T]
    in_maps = []
    for c in range(N_CORES):
        h0 = c * HPC
        cols = []
        for part in range(3):  # q, k, v blocks of Wqkv columns
            for h in range(HPC):
                j0 = part * C + (h0 + h) * HD
                cols.append(Wqkv[:, j0:j0 + HD])
        wq = np.ascontiguousarray(np.concatenate(cols, axis=1)).astype(np.float16)
        wp = np.ascontiguousarray(
            Wproj[h0 * HD:(h0 + HPC) * HD, :]).astype(np.float16)
        in_maps.append({"xt": xT, "wqkv": wq, "wproj": wp})
    return in_maps


def run_shards(in_maps, trace=False):
    from concourse.bass_utils import run_bass_kernel_spmd
    nc = _get_nc()
    last_err = None
    for _attempt in range(3):
        try:
            return run_bass_kernel_spmd(
                nc, in_maps, core_ids=list(range(N_CORES)), trace=trace)
        except Exception as e:  # transient NRT device errors — retry
            last_err = e
            if "UNAVAILABLE" not in str(e) and "UNRECOVERABLE" not in str(e):
                raise
    raise last_err


def kernel(x, Wqkv, Wproj):
    x = np.asarray(x, dtype=np.float32)
    Wqkv = np.asarray(Wqkv, dtype=np.float32)
    Wproj = np.asarray(Wproj, dtype=np.float32)
    x2d = np.ascontiguousarray(x.reshape(B * T, C))

    in_maps = _make_in_maps(x2d, Wqkv, Wproj)
    res = run_shards(in_maps)

    acc = res.results[0]["out"].astype(np.float32)
    for c in range(1, N_CORES):
        acc += res.results[c]["out"].astype(np.float32)
    return acc.reshape(B, T, C)
